# revision 1
# baseline (speedup 1.0000x reference)
"""CRF layer (forward-algorithm NLL) on 8 Trainium2 NeuronCores.

Strategy
--------
Data-parallel over the batch: 8 cores x 32 sequences. Per core the
log-partition logZ is computed in *probability space*:

    p_{t+1} = diag(exp(x_t)) @ exp(trans) @ p_t

which maps to one 128x128xN matmul (stationary exp(trans), N=32
sequences) plus one elementwise multiply (DVE tensor_tensor reading the
matmul's PSUM output) per timestep. The scan is serial, so we run TWO
independent chains per core concurrently to hide the per-step latency:
a forward chain over t=0..511 and a backward (beta) chain over
t=1023..512; they meet in the middle and combine via a dot product.

Emissions are pre-transposed and cast to bf16 on the host (a
layout/sharding choice) so the device streams them directly in
[tag, (t, seq)] order; exp() runs on the scalar engine in bulk.
Periodic renormalization (every 32 steps) keeps magnitudes in range:
Z is computed+broadcast with a ones-matmul, the reciprocal scales the
next emission slice, and log Z accumulates into a running correction.

The gold-path score (a simple gather/sum, O(B*L)) is computed on the
host in float64. Output: nll[256] float32.
"""

import numpy as np
import ml_dtypes

B, L, NTAG = 256, 1024, 128
NCORES = 8
SEQ = B // NCORES          # 32 sequences per core
NF = 512                   # forward steps; backward covers the rest
NB = L - NF
CH = 64                    # timesteps per emission chunk
RENORM = 32                # renormalize every RENORM steps
START, END = 126, 127
LNS = float(np.log(128.0) + 0.5)   # per-step prescale: exp(trans) * e^-LNS

_PROG = None               # cached compiled program


def _build_program():
    from contextlib import ExitStack

    import concourse.bacc as bacc
    import concourse.bass as bass
    import concourse.tile as tile
    import concourse.mybir as mybir
    from concourse.alu_op_type import AluOpType

    F32 = mybir.dt.float32
    BF16 = mybir.dt.bfloat16
    MULT = AluOpType.mult
    ADD = AluOpType.add

    nc = bacc.Bacc("TRN2", target_bir_lowering=False, debug=False)

    XT = nc.dram_tensor("XT", (NTAG, L, SEQ), BF16, kind="ExternalInput")
    EF = nc.dram_tensor("EF", (NTAG, NTAG), BF16, kind="ExternalInput")
    EB = nc.dram_tensor("EB", (NTAG, NTAG), BF16, kind="ExternalInput")
    PINIT = nc.dram_tensor("PINIT", (NTAG, SEQ), BF16, kind="ExternalInput")
    BINIT = nc.dram_tensor("BINIT", (NTAG, SEQ), F32, kind="ExternalInput")
    OUT = nc.dram_tensor("OUT", (3, NTAG, SEQ), F32, kind="ExternalOutput")

    with tile.TileContext(nc) as tc, ExitStack() as ctx:
        const = ctx.enter_context(tc.tile_pool(name="const", bufs=1))
        xpool = ctx.enter_context(tc.tile_pool(name="xchunk", bufs=2))
        epool = ctx.enter_context(tc.tile_pool(name="echunk", bufs=2))
        spool = ctx.enter_context(tc.tile_pool(name="state", bufs=3))
        rpool = ctx.enter_context(tc.tile_pool(name="renorm", bufs=2))
        qpool = ctx.enter_context(tc.tile_pool(name="qpsum", bufs=2, space="PSUM"))
        zpool = ctx.enter_context(tc.tile_pool(name="zpsum", bufs=2, space="PSUM"))

        ef = const.tile([NTAG, NTAG], BF16, tag="ef")
        nc.sync.dma_start(ef[:], EF[:])
        eb = const.tile([NTAG, NTAG], BF16, tag="eb")
        nc.sync.dma_start(eb[:], EB[:])
        ones = const.tile([NTAG, NTAG], BF16, tag="ones")
        nc.gpsimd.memset(ones[:], 1.0)
        p0 = const.tile([NTAG, SEQ], BF16, tag="p0")
        nc.sync.dma_start(p0[:], PINIT[:])
        b0 = const.tile([NTAG, SEQ], F32, tag="b0")
        nc.sync.dma_start(b0[:], BINIT[:])
        cf = const.tile([NTAG, SEQ], F32, tag="cf")
        nc.gpsimd.memset(cf[:], 0.0)
        cb = const.tile([NTAG, SEQ], F32, tag="cb")
        nc.gpsimd.memset(cb[:], 0.0)

        EXP = mybir.ActivationFunctionType.Exp
        LN = mybir.ActivationFunctionType.Ln

        nchunks = L // CH

        def load_chunk(kc, dirtag):
            xc = xpool.tile([NTAG, CH * SEQ], BF16, tag=f"x{dirtag}")
            nc.sync.dma_start(
                xc[:], XT[:, kc * CH:(kc + 1) * CH, :].rearrange("p t s -> p (t s)")
            )
            ec = epool.tile([NTAG, CH * SEQ], F32, tag=f"e{dirtag}")
            nc.scalar.activation(ec[:], xc[:], EXP)
            return ec

        # chain state
        pf = p0                 # fwd state (SBUF bf16), updated per step
        bb = None               # bwd state (PSUM f32 after first MM)
        ub = None
        ef_chunk = None
        eb_chunk = None
        rzf = rzb = None        # pending renorm reciprocal scales

        for w in range(NF):
            tf = w              # fwd timestep
            tb = L - 1 - w      # bwd timestep

            if w % CH == 0:
                ef_chunk = load_chunk(tf // CH, "f")
                eb_chunk = load_chunk(tb // CH, "b")

            # ---- renorm (every RENORM steps, not at w=0) ----
            if w % RENORM == 0 and w > 0:
                zf = zpool.tile([NTAG, SEQ], F32, tag="zf")
                nc.tensor.matmul(zf[:], ones[:], pf[:], start=True, stop=True)
                rzf = rpool.tile([NTAG, SEQ], F32, tag="rzf")
                nc.vector.reciprocal(rzf[:], zf[:])
                lnzf = rpool.tile([NTAG, SEQ], F32, tag="lnzf")
                nc.scalar.activation(lnzf[:], zf[:], LN)
                nc.vector.tensor_tensor(cf[:], cf[:], lnzf[:], ADD)

                zb = zpool.tile([NTAG, SEQ], F32, tag="zb")
                nc.tensor.matmul(zb[:], ones[:], ub[:], start=True, stop=True)
                rzb = rpool.tile([NTAG, SEQ], F32, tag="rzb")
                nc.vector.reciprocal(rzb[:], zb[:])
                lnzb = rpool.tile([NTAG, SEQ], F32, tag="lnzb")
                nc.scalar.activation(lnzb[:], zb[:], LN)
                nc.vector.tensor_tensor(cb[:], cb[:], lnzb[:], ADD)

            # ---- forward step ----
            lf = tf % CH
            e_f = ef_chunk[:, lf * SEQ:(lf + 1) * SEQ]
            if rzf is not None:
                e2 = rpool.tile([NTAG, SEQ], F32, tag="e2f")
                nc.vector.tensor_tensor(e2[:], e_f, rzf[:], MULT)
                e_f = e2[:]
                rzf = None
            qf = qpool.tile([NTAG, SEQ], F32, tag="qf")
            nc.tensor.matmul(qf[:], ef[:], pf[:], start=True, stop=True)
            last = w == NF - 1
            pn = spool.tile([NTAG, SEQ], F32 if last else BF16,
                            tag="pflast" if last else "pf")
            nc.vector.tensor_tensor(pn[:], qf[:], e_f, MULT)
            pf = pn

            # ---- backward step ----
            lb = tb % CH
            e_b = eb_chunk[:, lb * SEQ:(lb + 1) * SEQ]
            if rzb is not None:
                e2 = rpool.tile([NTAG, SEQ], F32, tag="e2b")
                nc.vector.tensor_tensor(e2[:], e_b, rzb[:], MULT)
                e_b = e2[:]
                rzb = None
            un = spool.tile([NTAG, SEQ], BF16, tag="ub")
            src = b0[:] if bb is None else bb[:]
            nc.vector.tensor_tensor(un[:], src, e_b, MULT)
            ub = un
            bb = qpool.tile([NTAG, SEQ], F32, tag="bb")
            nc.tensor.matmul(bb[:], eb[:], ub[:], start=True, stop=True)

        # ---- junction: D_elem = pf * bb, shipped to host ----
        delem = spool.tile([NTAG, SEQ], F32, tag="delem")
        nc.vector.tensor_tensor(delem[:], bb[:], pf[:], MULT)
        nc.sync.dma_start(OUT[0, :, :], delem[:])
        nc.sync.dma_start(OUT[1, :, :], cf[:])
        nc.sync.dma_start(OUT[2, :, :], cb[:])

    nc.compile()
    return nc


def _get_program():
    global _PROG
    if _PROG is None:
        _PROG = _build_program()
    return _PROG


def _gold_score(X, y, trans):
    """Gold path score per sequence, float64 on host."""
    Xd = X.astype(np.float64)
    td = trans.astype(np.float64)
    yi = y.astype(np.int64)
    prev = np.concatenate(
        [np.full((B, 1), START, dtype=np.int64), yi[:, :-1]], axis=1
    )
    emit = np.take_along_axis(Xd, yi[:, :, None], axis=2)[:, :, 0]  # [B, L]
    tr = td[yi, prev]                                               # [B, L]
    return emit.sum(1) + tr.sum(1) + td[END, yi[:, -1]]


def kernel(X, y, trans):
    from concourse import bass_utils

    nc = _get_program()

    bf16 = ml_dtypes.bfloat16
    Xb = X.astype(bf16)
    efm = np.exp(trans.astype(np.float64).T - LNS).astype(bf16)   # [j, i]
    ebm = np.exp(trans.astype(np.float64) - LNS).astype(bf16)     # [i, j]
    pinit = np.zeros((NTAG, SEQ), dtype=bf16)
    pinit[START, :] = 1.0
    binit = np.repeat(
        np.exp(trans[END, :].astype(np.float64) - LNS)[:, None], SEQ, axis=1
    ).astype(np.float32)

    in_maps = []
    for c in range(NCORES):
        xt = np.ascontiguousarray(Xb[c * SEQ:(c + 1) * SEQ].transpose(2, 1, 0))
        in_maps.append(
            {"XT": xt, "EF": efm, "EB": ebm, "PINIT": pinit, "BINIT": binit}
        )

    res = bass_utils.run_bass_kernel_spmd(
        nc, in_maps, core_ids=list(range(NCORES))
    )

    logZ = np.empty(B, dtype=np.float64)
    for c in range(NCORES):
        out = res.results[c]["OUT"].astype(np.float64)
        dot = out[0].sum(axis=0)
        logZ[c * SEQ:(c + 1) * SEQ] = (
            np.log(dot) + out[1][0] + out[2][0] + (L + 1) * LNS
        )

    gold = _gold_score(X, y, trans)
    return (logZ - gold).astype(np.float32)


# revision 10
# speedup vs baseline: 1.0909x; 1.0909x over previous
"""CRF layer (forward-algorithm NLL) on 8 Trainium2 NeuronCores.

Strategy
--------
Data-parallel over the batch: 8 cores x 32 sequences. Per core the
log-partition logZ is computed in *probability space*:

    p_{t+1} = diag(exp(x_t)) @ exp(trans) @ p_t

which maps to one 128x128xN matmul (stationary exp(trans), N=32
sequences) plus one elementwise multiply (DVE tensor_tensor reading the
matmul's PSUM output) per timestep. The scan is serial, so we run TWO
independent chains per core concurrently to hide the per-step latency:
a forward chain over t=0..511 and a backward (beta) chain over
t=1023..512; they meet in the middle and combine via a dot product.

Emissions are pre-transposed and cast to bf16 on the host (a
layout/sharding choice) so the device streams them directly in
[tag, (t, seq)] order; exp() runs on the scalar engine in bulk.
Periodic renormalization (every 32 steps) keeps magnitudes in range:
Z is computed+broadcast with a ones-matmul, the reciprocal scales the
next emission slice, and log Z accumulates into a running correction.

The gold-path score (a simple gather/sum, O(B*L)) is computed on the
host in float64. Output: nll[256] float32.
"""

import numpy as np
import ml_dtypes

B, L, NTAG = 256, 1024, 128
NCORES = 8
SEQ = B // NCORES          # 32 sequences per core
NF = 512                   # forward steps; backward covers the rest
NB = L - NF
CH = 64                    # timesteps per emission chunk
RENORM = 64                # renormalize every RENORM steps
NRE = NF // RENORM - 1     # renorm events per chain (none at w=0)
E_BF16 = True              # exp() output dtype for emission tiles
START, END = 126, 127
LNS = float(np.log(128.0) + 0.5)   # per-step prescale: exp(trans) * e^-LNS

_PROG = None               # cached compiled program


def _build_program():
    from contextlib import ExitStack

    import concourse.bacc as bacc
    import concourse.bass as bass
    import concourse.tile as tile
    import concourse.mybir as mybir
    from concourse.alu_op_type import AluOpType

    F32 = mybir.dt.float32
    BF16 = mybir.dt.bfloat16
    MULT = AluOpType.mult
    ADD = AluOpType.add

    nc = bacc.Bacc("TRN2", target_bir_lowering=False, debug=False)

    XT = nc.dram_tensor("XT", (NTAG, L, SEQ), BF16, kind="ExternalInput")
    EF = nc.dram_tensor("EF", (NTAG, NTAG), BF16, kind="ExternalInput")
    EB = nc.dram_tensor("EB", (NTAG, NTAG), BF16, kind="ExternalInput")
    PINIT = nc.dram_tensor("PINIT", (NTAG, SEQ), BF16, kind="ExternalInput")
    BINIT = nc.dram_tensor("BINIT", (NTAG, SEQ), F32, kind="ExternalInput")
    OUT = nc.dram_tensor("OUT", (NTAG, SEQ), F32, kind="ExternalOutput")
    RZF = nc.dram_tensor("RZF", (NTAG, NRE * SEQ), F32, kind="ExternalOutput")
    RZB = nc.dram_tensor("RZB", (NTAG, NRE * SEQ), F32, kind="ExternalOutput")

    with tile.TileContext(nc) as tc, ExitStack() as ctx:
        const = ctx.enter_context(tc.tile_pool(name="const", bufs=1))
        xpool = ctx.enter_context(tc.tile_pool(name="xchunk", bufs=2))
        epool = ctx.enter_context(tc.tile_pool(name="echunk", bufs=2))
        spool = ctx.enter_context(tc.tile_pool(name="state", bufs=3))
        rpool = ctx.enter_context(tc.tile_pool(name="renorm", bufs=2))
        qpool = ctx.enter_context(tc.tile_pool(name="qpsum", bufs=2, space="PSUM"))
        zpool = ctx.enter_context(tc.tile_pool(name="zpsum", bufs=2, space="PSUM"))

        ef = const.tile([NTAG, NTAG], BF16, tag="ef")
        nc.sync.dma_start(ef[:], EF[:])
        eb = const.tile([NTAG, NTAG], BF16, tag="eb")
        nc.sync.dma_start(eb[:], EB[:])
        ones = const.tile([NTAG, NTAG], BF16, tag="ones")
        nc.gpsimd.memset(ones[:], 1.0)
        p0 = const.tile([NTAG, SEQ], BF16, tag="p0")
        nc.sync.dma_start(p0[:], PINIT[:])
        b0 = const.tile([NTAG, SEQ], F32, tag="b0")
        nc.sync.dma_start(b0[:], BINIT[:])
        rzbuf_f = const.tile([NTAG, NRE * SEQ], F32, tag="rzbf")
        rzbuf_b = const.tile([NTAG, NRE * SEQ], F32, tag="rzbb")

        EXP = mybir.ActivationFunctionType.Exp
        EDT = BF16 if E_BF16 else F32

        def load_chunk(kc, dirtag):
            xc = xpool.tile([NTAG, CH * SEQ], BF16, tag=f"x{dirtag}")
            nc.sync.dma_start(
                xc[:], XT[:, kc * CH:(kc + 1) * CH, :].rearrange("p t s -> p (t s)")
            )
            ec = epool.tile([NTAG, CH * SEQ], EDT, tag=f"e{dirtag}")
            nc.scalar.activation(ec[:], xc[:], EXP)
            return ec

        # chain state
        pf = p0                 # fwd state (SBUF bf16), updated per step
        bb = None               # bwd state (PSUM f32 after first MM)
        ub = None
        ef_chunk = None
        eb_chunk = None
        rzf = rzb = None        # pending renorm reciprocal scales

        for w in range(NF):
            tf = w              # fwd timestep
            tb = L - 1 - w      # bwd timestep

            if w % CH == 0:
                ef_chunk = load_chunk(tf // CH, "f")
                eb_chunk = load_chunk(tb // CH, "b")

            # ---- renorm (every RENORM steps, not at w=0) ----
            if w % RENORM == 0 and w > 0:
                kre = w // RENORM - 1
                sl = slice(kre * SEQ, (kre + 1) * SEQ)
                zf = zpool.tile([NTAG, SEQ], F32, tag="zf")
                nc.tensor.matmul(zf[:], ones[:], pf[:], start=True, stop=True)
                rzf = rzbuf_f[:, sl]
                nc.vector.reciprocal(rzf, zf[:])

                zb = zpool.tile([NTAG, SEQ], F32, tag="zb")
                nc.tensor.matmul(zb[:], ones[:], ub[:], start=True, stop=True)
                rzb = rzbuf_b[:, sl]
                nc.vector.reciprocal(rzb, zb[:])

            # ---- forward step ----
            lf = tf % CH
            e_f = ef_chunk[:, lf * SEQ:(lf + 1) * SEQ]
            if rzf is not None:
                e2 = rpool.tile([NTAG, SEQ], EDT, tag="e2f")
                nc.vector.tensor_tensor(e2[:], e_f, rzf, MULT)
                e_f = e2[:]
                rzf = None
            qf = qpool.tile([NTAG, SEQ], F32, tag="qf")
            nc.tensor.matmul(qf[:], ef[:], pf[:], start=True, stop=True)
            last = w == NF - 1
            pn = spool.tile([NTAG, SEQ], F32 if last else BF16,
                            tag="pflast" if last else "pf")
            nc.vector.tensor_tensor(pn[:], qf[:], e_f, MULT)
            pf = pn

            # ---- backward step ----
            lb = tb % CH
            e_b = eb_chunk[:, lb * SEQ:(lb + 1) * SEQ]
            if rzb is not None:
                e2 = rpool.tile([NTAG, SEQ], EDT, tag="e2b")
                nc.vector.tensor_tensor(e2[:], e_b, rzb, MULT)
                e_b = e2[:]
                rzb = None
            un = spool.tile([NTAG, SEQ], BF16, tag="ub")
            src = b0[:] if bb is None else bb[:]
            nc.vector.tensor_tensor(un[:], src, e_b, MULT)
            ub = un
            bb = qpool.tile([NTAG, SEQ], F32, tag="bb")
            nc.tensor.matmul(bb[:], eb[:], ub[:], start=True, stop=True)

        # ---- junction: D_elem = pf * bb, shipped to host ----
        delem = spool.tile([NTAG, SEQ], F32, tag="delem")
        nc.vector.tensor_tensor(delem[:], bb[:], pf[:], MULT)
        nc.sync.dma_start(OUT[:], delem[:])
        nc.sync.dma_start(RZF[:], rzbuf_f[:])
        nc.sync.dma_start(RZB[:], rzbuf_b[:])

    nc.compile()
    return nc


def _get_program():
    global _PROG
    if _PROG is None:
        _PROG = _build_program()
    return _PROG


def _gold_score(X, y, trans):
    """Gold path score per sequence, float64 on host."""
    Xd = X.astype(np.float64)
    td = trans.astype(np.float64)
    yi = y.astype(np.int64)
    prev = np.concatenate(
        [np.full((B, 1), START, dtype=np.int64), yi[:, :-1]], axis=1
    )
    emit = np.take_along_axis(Xd, yi[:, :, None], axis=2)[:, :, 0]  # [B, L]
    tr = td[yi, prev]                                               # [B, L]
    return emit.sum(1) + tr.sum(1) + td[END, yi[:, -1]]


def _prep_in_maps(X, trans):
    bf16 = ml_dtypes.bfloat16
    Xb = X.astype(bf16)
    efm = np.exp(trans.astype(np.float64).T - LNS).astype(bf16)   # [j, i]
    ebm = np.exp(trans.astype(np.float64) - LNS).astype(bf16)     # [i, j]
    pinit = np.zeros((NTAG, SEQ), dtype=bf16)
    pinit[START, :] = 1.0
    binit = np.repeat(
        np.exp(trans[END, :].astype(np.float64) - LNS)[:, None], SEQ, axis=1
    ).astype(np.float32)

    in_maps = []
    for c in range(NCORES):
        xt = np.ascontiguousarray(Xb[c * SEQ:(c + 1) * SEQ].transpose(2, 1, 0))
        in_maps.append(
            {"XT": xt, "EF": efm, "EB": ebm, "PINIT": pinit, "BINIT": binit}
        )
    return in_maps


def kernel(X, y, trans):
    from concourse import bass_utils

    nc = _get_program()
    in_maps = _prep_in_maps(X, trans)
    res = bass_utils.run_bass_kernel_spmd(
        nc, in_maps, core_ids=list(range(NCORES))
    )

    logZ = np.empty(B, dtype=np.float64)
    for c in range(NCORES):
        r = res.results[c]
        dot = r["OUT"].astype(np.float64).sum(axis=0)
        # rz rows are identical across partitions; row 0 has the values
        rzf = r["RZF"][0].astype(np.float64).reshape(NRE, SEQ)
        rzb = r["RZB"][0].astype(np.float64).reshape(NRE, SEQ)
        cf = -np.log(rzf).sum(axis=0)
        cb = -np.log(rzb).sum(axis=0)
        logZ[c * SEQ:(c + 1) * SEQ] = np.log(dot) + cf + cb + (L + 1) * LNS

    gold = _gold_score(X, y, trans)
    return (logZ - gold).astype(np.float32)


# revision 17
# speedup vs baseline: 1.1019x; 1.0101x over previous
"""CRF layer (forward-algorithm NLL) on 8 Trainium2 NeuronCores.

Strategy
--------
Data-parallel over the batch: 8 cores x 32 sequences. Per core the
log-partition logZ is computed in *probability space*:

    p_{t+1} = diag(exp(x_t)) @ exp(trans) @ p_t

which maps to one 128x128xN matmul (stationary exp(trans), N=32
sequences) plus one elementwise multiply (DVE tensor_tensor reading the
matmul's PSUM output) per timestep. The scan is serial, so we run TWO
independent chains per core concurrently to hide the per-step latency:
a forward chain over t=0..511 and a backward (beta) chain over
t=1023..512; they meet in the middle and combine via a dot product.

Emissions are pre-transposed and cast to bf16 on the host (a
layout/sharding choice) so the device streams them directly in
[tag, (t, seq)] order; exp() runs on the scalar engine in bulk.
Periodic renormalization (every 32 steps) keeps magnitudes in range:
Z is computed+broadcast with a ones-matmul, the reciprocal scales the
next emission slice, and log Z accumulates into a running correction.

The gold-path score (a simple gather/sum, O(B*L)) is computed on the
host in float64. Output: nll[256] float32.
"""

import numpy as np
import ml_dtypes

B, L, NTAG = 256, 1024, 128
NCORES = 8
SEQ = B // NCORES          # 32 sequences per core
NF = 512                   # forward steps; backward covers the rest
NB = L - NF
CH = 64                    # timesteps per emission chunk
RENORM = 64                # renormalize every RENORM steps
NRE = NF // RENORM - 1     # renorm events per chain (none at w=0)
E_BF16 = True              # exp() output dtype for emission tiles
START, END = 126, 127
LNS = float(np.log(128.0) + 0.5)   # per-step prescale: exp(trans) * e^-LNS

_PROG = None               # cached compiled program


def _build_program():
    from contextlib import ExitStack

    import concourse.bacc as bacc
    import concourse.bass as bass
    import concourse.tile as tile
    import concourse.mybir as mybir
    from concourse.alu_op_type import AluOpType

    F32 = mybir.dt.float32
    BF16 = mybir.dt.bfloat16
    MULT = AluOpType.mult
    ADD = AluOpType.add

    nc = bacc.Bacc("TRN2", target_bir_lowering=False, debug=False)

    XT = nc.dram_tensor("XT", (NTAG, L, SEQ), BF16, kind="ExternalInput")
    EF = nc.dram_tensor("EF", (NTAG, NTAG), BF16, kind="ExternalInput")
    EB = nc.dram_tensor("EB", (NTAG, NTAG), BF16, kind="ExternalInput")
    PINIT = nc.dram_tensor("PINIT", (NTAG, SEQ), BF16, kind="ExternalInput")
    BINIT = nc.dram_tensor("BINIT", (NTAG, SEQ), F32, kind="ExternalInput")
    OUT = nc.dram_tensor("OUT", (NTAG, SEQ), F32, kind="ExternalOutput")
    RZF = nc.dram_tensor("RZF", (NTAG, NRE * SEQ), F32, kind="ExternalOutput")
    RZB = nc.dram_tensor("RZB", (NTAG, NRE * SEQ), F32, kind="ExternalOutput")

    with tile.TileContext(nc) as tc, ExitStack() as ctx:
        const = ctx.enter_context(tc.tile_pool(name="const", bufs=1))
        xpool = ctx.enter_context(tc.tile_pool(name="xchunk", bufs=3))
        epool = ctx.enter_context(tc.tile_pool(name="echunk", bufs=3))
        spool = ctx.enter_context(tc.tile_pool(name="state", bufs=3))
        rpool = ctx.enter_context(tc.tile_pool(name="renorm", bufs=2))
        qpool = ctx.enter_context(tc.tile_pool(name="qpsum", bufs=2, space="PSUM"))
        zpool = ctx.enter_context(tc.tile_pool(name="zpsum", bufs=1, space="PSUM"))

        ef = const.tile([NTAG, NTAG], BF16, tag="ef")
        nc.sync.dma_start(ef[:], EF[:])
        eb = const.tile([NTAG, NTAG], BF16, tag="eb")
        nc.sync.dma_start(eb[:], EB[:])
        ones = const.tile([NTAG, NTAG], BF16, tag="ones")
        nc.gpsimd.memset(ones[:], 1.0)
        p0 = const.tile([NTAG, SEQ], BF16, tag="p0")
        nc.sync.dma_start(p0[:], PINIT[:])
        b0 = const.tile([NTAG, SEQ], F32, tag="b0")
        nc.sync.dma_start(b0[:], BINIT[:])
        rzbuf_f = const.tile([NTAG, NRE * SEQ], F32, tag="rzbf")
        rzbuf_b = const.tile([NTAG, NRE * SEQ], F32, tag="rzbb")

        EXP = mybir.ActivationFunctionType.Exp
        EDT = BF16 if E_BF16 else F32

        class EStream:
            """Emission streamer for one chain: DMA + exp in ranges, with
            one-range lookahead so exp never blocks the recurrence."""

            def __init__(self, dirtag, ranges):
                self.dirtag = dirtag
                self.ranges = ranges       # list of (t0, nsteps)
                self.tiles = {}            # range idx -> (t0, n, tile)
                self.cur = 0
                self.pushed = 0

            def push(self):
                if self.pushed >= len(self.ranges):
                    return
                i = self.pushed
                t0, n = self.ranges[i]
                xc = xpool.tile([NTAG, n * SEQ], BF16, tag=f"x{self.dirtag}")
                nc.sync.dma_start(
                    xc[:],
                    XT[:, t0:t0 + n, :].rearrange("p t s -> p (t s)"),
                )
                ec = epool.tile([NTAG, n * SEQ], EDT, tag=f"e{self.dirtag}")
                nc.scalar.activation(ec[:], xc[:], EXP)
                self.tiles[i] = (t0, n, ec)
                self.pushed += 1

            def slice(self, t):
                t0, n, ec = self.tiles[self.cur]
                if not (t0 <= t < t0 + n):
                    del self.tiles[self.cur]
                    self.cur += 1
                    self.push()
                    t0, n, ec = self.tiles[self.cur]
                assert t0 <= t < t0 + n, (t, t0, n)
                lt = t - t0
                return ec[:, lt * SEQ:(lt + 1) * SEQ]

        def ranges_fwd():
            r = [(0, 8), (8, CH - 8)]
            for t0 in range(CH, NF, CH):
                r.append((t0, CH))
            return r

        def ranges_bwd():
            r = [(L - 8, 8), (L - CH, CH - 8)]
            for t0 in range(L - 2 * CH, NF - 1, -CH):
                r.append((t0, CH))
            return r

        sf = EStream("f", ranges_fwd())
        sb = EStream("b", ranges_bwd())
        sf.push(), sf.push()
        sb.push(), sb.push()

        # chain state
        pf = p0                 # fwd state (SBUF bf16), updated per step
        bb = None               # bwd state (PSUM f32 after first MM)
        ub = None
        rzf = rzb = None        # pending renorm reciprocal scales

        for w in range(NF):
            tf = w              # fwd timestep
            tb = L - 1 - w      # bwd timestep

            # ---- renorm (every RENORM steps, not at w=0) ----
            if w % RENORM == 0 and w > 0:
                kre = w // RENORM - 1
                sl = slice(kre * SEQ, (kre + 1) * SEQ)
                zf = zpool.tile([NTAG, SEQ], F32, tag="zf")
                nc.tensor.matmul(zf[:], ones[:], pf[:], start=True, stop=True)
                rzf = rzbuf_f[:, sl]
                nc.vector.reciprocal_approx_fast(out=rzf, in_=zf[:])

                zb = zpool.tile([NTAG, SEQ], F32, tag="zb")
                nc.tensor.matmul(zb[:], ones[:], ub[:], start=True, stop=True)
                rzb = rzbuf_b[:, sl]
                nc.vector.reciprocal_approx_fast(out=rzb, in_=zb[:])

            # ---- backward step (emitted first: its TT reads old PSUM) ----
            e_b = sb.slice(tb)
            if rzb is not None:
                e2 = rpool.tile([NTAG, SEQ], EDT, tag="e2b")
                nc.vector.tensor_tensor(e2[:], e_b, rzb, MULT)
                e_b = e2[:]
                rzb = None
            un = spool.tile([NTAG, SEQ], BF16, tag="ub")
            src = b0[:] if bb is None else bb[:]
            nc.vector.tensor_tensor(un[:], src, e_b, MULT)
            ub = un
            bb = qpool.tile([NTAG, SEQ], F32, tag="bb")
            nc.tensor.matmul(bb[:], eb[:], ub[:], start=True, stop=True)

            # ---- forward step ----
            e_f = sf.slice(tf)
            if rzf is not None:
                e2 = rpool.tile([NTAG, SEQ], EDT, tag="e2f")
                nc.vector.tensor_tensor(e2[:], e_f, rzf, MULT)
                e_f = e2[:]
                rzf = None
            qf = qpool.tile([NTAG, SEQ], F32, tag="qf")
            nc.tensor.matmul(qf[:], ef[:], pf[:], start=True, stop=True)
            last = w == NF - 1
            pn = spool.tile([NTAG, SEQ], F32 if last else BF16,
                            tag="pflast" if last else "pf")
            nc.vector.tensor_tensor(pn[:], qf[:], e_f, MULT)
            pf = pn

        # ---- junction: D_elem = pf * bb, shipped to host ----
        delem = spool.tile([NTAG, SEQ], F32, tag="delem")
        nc.vector.tensor_tensor(delem[:], bb[:], pf[:], MULT)
        nc.sync.dma_start(OUT[:], delem[:])
        nc.sync.dma_start(RZF[:], rzbuf_f[:])
        nc.sync.dma_start(RZB[:], rzbuf_b[:])

    nc.compile()
    return nc


def _get_program():
    global _PROG
    if _PROG is None:
        _PROG = _build_program()
    return _PROG


def _gold_score(X, y, trans):
    """Gold path score per sequence, float64 on host."""
    Xd = X.astype(np.float64)
    td = trans.astype(np.float64)
    yi = y.astype(np.int64)
    prev = np.concatenate(
        [np.full((B, 1), START, dtype=np.int64), yi[:, :-1]], axis=1
    )
    emit = np.take_along_axis(Xd, yi[:, :, None], axis=2)[:, :, 0]  # [B, L]
    tr = td[yi, prev]                                               # [B, L]
    return emit.sum(1) + tr.sum(1) + td[END, yi[:, -1]]


def _prep_in_maps(X, trans):
    bf16 = ml_dtypes.bfloat16
    Xb = X.astype(bf16)
    efm = np.exp(trans.astype(np.float64).T - LNS).astype(bf16)   # [j, i]
    ebm = np.exp(trans.astype(np.float64) - LNS).astype(bf16)     # [i, j]
    pinit = np.zeros((NTAG, SEQ), dtype=bf16)
    pinit[START, :] = 1.0
    binit = np.repeat(
        np.exp(trans[END, :].astype(np.float64) - LNS)[:, None], SEQ, axis=1
    ).astype(np.float32)

    in_maps = []
    for c in range(NCORES):
        xt = np.ascontiguousarray(Xb[c * SEQ:(c + 1) * SEQ].transpose(2, 1, 0))
        in_maps.append(
            {"XT": xt, "EF": efm, "EB": ebm, "PINIT": pinit, "BINIT": binit}
        )
    return in_maps


def kernel(X, y, trans):
    from concourse import bass_utils

    nc = _get_program()
    in_maps = _prep_in_maps(X, trans)
    res = bass_utils.run_bass_kernel_spmd(
        nc, in_maps, core_ids=list(range(NCORES))
    )

    logZ = np.empty(B, dtype=np.float64)
    for c in range(NCORES):
        r = res.results[c]
        dot = r["OUT"].astype(np.float64).sum(axis=0)
        # rz rows are identical across partitions; row 0 has the values
        rzf = r["RZF"][0].astype(np.float64).reshape(NRE, SEQ)
        rzb = r["RZB"][0].astype(np.float64).reshape(NRE, SEQ)
        cf = -np.log(rzf).sum(axis=0)
        cb = -np.log(rzb).sum(axis=0)
        logZ[c * SEQ:(c + 1) * SEQ] = np.log(dot) + cf + cb + (L + 1) * LNS

    gold = _gold_score(X, y, trans)
    return (logZ - gold).astype(np.float32)


# revision 18
# speedup vs baseline: 1.1024x; 1.0005x over previous
"""CRF layer (forward-algorithm NLL) on 8 Trainium2 NeuronCores.

Strategy
--------
Data-parallel over the batch: 8 cores x 32 sequences. Per core the
log-partition logZ is computed in *probability space*:

    p_{t+1} = diag(exp(x_t)) @ exp(trans) @ p_t

which maps to one 128x128xN matmul (stationary exp(trans), N=32
sequences) plus one elementwise multiply (DVE tensor_tensor reading the
matmul's PSUM output) per timestep. The scan is serial, so we run TWO
independent chains per core concurrently to hide the per-step latency:
a forward chain over t=0..511 and a backward (beta) chain over
t=1023..512; they meet in the middle and combine via a dot product.

Emissions are pre-transposed and cast to bf16 on the host (a
layout/sharding choice) so the device streams them directly in
[tag, (t, seq)] order; exp() runs on the scalar engine in bulk.
Periodic renormalization (every 32 steps) keeps magnitudes in range:
Z is computed+broadcast with a ones-matmul, the reciprocal scales the
next emission slice, and log Z accumulates into a running correction.

The gold-path score (a simple gather/sum, O(B*L)) is computed on the
host in float64. Output: nll[256] float32.
"""

import numpy as np
import ml_dtypes

B, L, NTAG = 256, 1024, 128
NCORES = 8
SEQ = B // NCORES          # 32 sequences per core
NF = 512                   # forward steps; backward covers the rest
NB = L - NF
CH = 64                    # timesteps per emission chunk
RENORM = 64                # renormalize every RENORM steps
NRE = NF // RENORM - 1     # renorm events per chain (none at w=0)
E_BF16 = True              # exp() output dtype for emission tiles
START, END = 126, 127
LNS = float(np.log(128.0) + 0.5)   # per-step prescale: exp(trans) * e^-LNS

_PROG = None               # cached compiled program


def _build_program():
    from contextlib import ExitStack

    import concourse.bacc as bacc
    import concourse.bass as bass
    import concourse.tile as tile
    import concourse.mybir as mybir
    from concourse.alu_op_type import AluOpType

    F32 = mybir.dt.float32
    BF16 = mybir.dt.bfloat16
    MULT = AluOpType.mult
    ADD = AluOpType.add

    nc = bacc.Bacc("TRN2", target_bir_lowering=False, debug=False)

    XT = nc.dram_tensor("XT", (NTAG, L, SEQ), BF16, kind="ExternalInput")
    EF = nc.dram_tensor("EF", (NTAG, NTAG), BF16, kind="ExternalInput")
    EB = nc.dram_tensor("EB", (NTAG, NTAG), BF16, kind="ExternalInput")
    PINIT = nc.dram_tensor("PINIT", (NTAG, SEQ), BF16, kind="ExternalInput")
    BINIT = nc.dram_tensor("BINIT", (NTAG, SEQ), F32, kind="ExternalInput")
    OUT = nc.dram_tensor("OUT", (NTAG, SEQ), F32, kind="ExternalOutput")
    RZF = nc.dram_tensor("RZF", (NTAG, NRE * SEQ), F32, kind="ExternalOutput")
    RZB = nc.dram_tensor("RZB", (NTAG, NRE * SEQ), F32, kind="ExternalOutput")

    with tile.TileContext(nc) as tc, ExitStack() as ctx:
        const = ctx.enter_context(tc.tile_pool(name="const", bufs=1))
        xpool = ctx.enter_context(tc.tile_pool(name="xchunk", bufs=3))
        epool = ctx.enter_context(tc.tile_pool(name="echunk", bufs=3))
        spool = ctx.enter_context(tc.tile_pool(name="state", bufs=3))
        rpool = ctx.enter_context(tc.tile_pool(name="renorm", bufs=2))
        qpool = ctx.enter_context(tc.tile_pool(name="qpsum", bufs=2, space="PSUM"))
        zpool = ctx.enter_context(tc.tile_pool(name="zpsum", bufs=1, space="PSUM"))

        ef = const.tile([NTAG, NTAG], BF16, tag="ef")
        nc.sync.dma_start(ef[:], EF[:])
        eb = const.tile([NTAG, NTAG], BF16, tag="eb")
        nc.sync.dma_start(eb[:], EB[:])
        ones = const.tile([NTAG, NTAG], BF16, tag="ones")
        nc.gpsimd.memset(ones[:], 1.0)
        p0 = const.tile([NTAG, SEQ], BF16, tag="p0")
        nc.sync.dma_start(p0[:], PINIT[:])
        b0 = const.tile([NTAG, SEQ], F32, tag="b0")
        nc.sync.dma_start(b0[:], BINIT[:])
        rzbuf_f = const.tile([NTAG, NRE * SEQ], F32, tag="rzbf")
        rzbuf_b = const.tile([NTAG, NRE * SEQ], F32, tag="rzbb")

        EXP = mybir.ActivationFunctionType.Exp
        EDT = BF16 if E_BF16 else F32

        class EStream:
            """Emission streamer for one chain: DMA + exp in ranges, with
            one-range lookahead so exp never blocks the recurrence."""

            def __init__(self, dirtag, ranges):
                self.dirtag = dirtag
                self.ranges = ranges       # list of (t0, nsteps)
                self.tiles = {}            # range idx -> (t0, n, tile)
                self.cur = 0
                self.pushed = 0

            def push(self):
                if self.pushed >= len(self.ranges):
                    return
                i = self.pushed
                t0, n = self.ranges[i]
                xc = xpool.tile([NTAG, n * SEQ], BF16, tag=f"x{self.dirtag}")
                nc.sync.dma_start(
                    xc[:],
                    XT[:, t0:t0 + n, :].rearrange("p t s -> p (t s)"),
                )
                ec = epool.tile([NTAG, n * SEQ], EDT, tag=f"e{self.dirtag}")
                nc.scalar.activation(ec[:], xc[:], EXP)
                self.tiles[i] = (t0, n, ec)
                self.pushed += 1

            def slice(self, t):
                t0, n, ec = self.tiles[self.cur]
                if not (t0 <= t < t0 + n):
                    del self.tiles[self.cur]
                    self.cur += 1
                    self.push()
                    t0, n, ec = self.tiles[self.cur]
                assert t0 <= t < t0 + n, (t, t0, n)
                lt = t - t0
                return ec[:, lt * SEQ:(lt + 1) * SEQ]

        def ranges_fwd():
            r = [(0, 8), (8, CH - 8)]
            for t0 in range(CH, NF, CH):
                r.append((t0, CH))
            return r

        def ranges_bwd():
            r = [(L - 8, 8), (L - CH, CH - 8)]
            for t0 in range(L - 2 * CH, NF - 1, -CH):
                r.append((t0, CH))
            return r

        sf = EStream("f", ranges_fwd())
        sb = EStream("b", ranges_bwd())
        sf.push(), sf.push()
        sb.push(), sb.push()

        # chain state
        pf = p0                 # fwd state (SBUF bf16), updated per step
        bb = None               # bwd state (PSUM f32 after first MM)
        ub = None
        rzf = rzb = None        # pending renorm reciprocal scales

        for w in range(NF):
            tf = w              # fwd timestep
            tb = L - 1 - w      # bwd timestep

            # ---- renorm (every RENORM steps, not at w=0) ----
            if w % RENORM == 0 and w > 0:
                kre = w // RENORM - 1
                sl = slice(kre * SEQ, (kre + 1) * SEQ)
                zf = zpool.tile([NTAG, SEQ], F32, tag="zf")
                nc.tensor.matmul(zf[:], ones[:], pf[:], start=True, stop=True)
                rzf = rzbuf_f[:, sl]
                nc.vector.reciprocal_approx_fast(out=rzf, in_=zf[:])

                zb = zpool.tile([NTAG, SEQ], F32, tag="zb")
                nc.tensor.matmul(zb[:], ones[:], ub[:], start=True, stop=True)
                rzb = rzbuf_b[:, sl]
                nc.vector.reciprocal_approx_fast(out=rzb, in_=zb[:])

            # ---- forward step ----
            e_f = sf.slice(tf)
            if rzf is not None:
                e2 = rpool.tile([NTAG, SEQ], EDT, tag="e2f")
                nc.vector.tensor_tensor(e2[:], e_f, rzf, MULT)
                e_f = e2[:]
                rzf = None
            qf = qpool.tile([NTAG, SEQ], F32, tag="qf")
            nc.tensor.matmul(qf[:], ef[:], pf[:], start=True, stop=True)
            last = w == NF - 1
            pn = spool.tile([NTAG, SEQ], F32 if last else BF16,
                            tag="pflast" if last else "pf")
            nc.vector.tensor_tensor(pn[:], qf[:], e_f, MULT)
            pf = pn

            # ---- backward step ----
            e_b = sb.slice(tb)
            if rzb is not None:
                e2 = rpool.tile([NTAG, SEQ], EDT, tag="e2b")
                nc.vector.tensor_tensor(e2[:], e_b, rzb, MULT)
                e_b = e2[:]
                rzb = None
            un = spool.tile([NTAG, SEQ], BF16, tag="ub")
            src = b0[:] if bb is None else bb[:]
            nc.vector.tensor_tensor(un[:], src, e_b, MULT)
            ub = un
            bb = qpool.tile([NTAG, SEQ], F32, tag="bb")
            nc.tensor.matmul(bb[:], eb[:], ub[:], start=True, stop=True)

        # ---- junction: D_elem = pf * bb, shipped to host ----
        delem = spool.tile([NTAG, SEQ], F32, tag="delem")
        nc.vector.tensor_tensor(delem[:], bb[:], pf[:], MULT)
        nc.sync.dma_start(OUT[:], delem[:])
        nc.sync.dma_start(RZF[:], rzbuf_f[:])
        nc.sync.dma_start(RZB[:], rzbuf_b[:])

    nc.compile()
    return nc


def _get_program():
    global _PROG
    if _PROG is None:
        _PROG = _build_program()
    return _PROG


def _gold_score(X, y, trans):
    """Gold path score per sequence, float64 on host."""
    Xd = X.astype(np.float64)
    td = trans.astype(np.float64)
    yi = y.astype(np.int64)
    prev = np.concatenate(
        [np.full((B, 1), START, dtype=np.int64), yi[:, :-1]], axis=1
    )
    emit = np.take_along_axis(Xd, yi[:, :, None], axis=2)[:, :, 0]  # [B, L]
    tr = td[yi, prev]                                               # [B, L]
    return emit.sum(1) + tr.sum(1) + td[END, yi[:, -1]]


def _prep_in_maps(X, trans):
    bf16 = ml_dtypes.bfloat16
    Xb = X.astype(bf16)
    efm = np.exp(trans.astype(np.float64).T - LNS).astype(bf16)   # [j, i]
    ebm = np.exp(trans.astype(np.float64) - LNS).astype(bf16)     # [i, j]
    pinit = np.zeros((NTAG, SEQ), dtype=bf16)
    pinit[START, :] = 1.0
    binit = np.repeat(
        np.exp(trans[END, :].astype(np.float64) - LNS)[:, None], SEQ, axis=1
    ).astype(np.float32)

    in_maps = []
    for c in range(NCORES):
        xt = np.ascontiguousarray(Xb[c * SEQ:(c + 1) * SEQ].transpose(2, 1, 0))
        in_maps.append(
            {"XT": xt, "EF": efm, "EB": ebm, "PINIT": pinit, "BINIT": binit}
        )
    return in_maps


def kernel(X, y, trans):
    from concourse import bass_utils

    nc = _get_program()
    in_maps = _prep_in_maps(X, trans)
    res = bass_utils.run_bass_kernel_spmd(
        nc, in_maps, core_ids=list(range(NCORES))
    )

    logZ = np.empty(B, dtype=np.float64)
    for c in range(NCORES):
        r = res.results[c]
        dot = r["OUT"].astype(np.float64).sum(axis=0)
        # rz rows are identical across partitions; row 0 has the values
        rzf = r["RZF"][0].astype(np.float64).reshape(NRE, SEQ)
        rzb = r["RZB"][0].astype(np.float64).reshape(NRE, SEQ)
        cf = -np.log(rzf).sum(axis=0)
        cb = -np.log(rzb).sum(axis=0)
        logZ[c * SEQ:(c + 1) * SEQ] = np.log(dot) + cf + cb + (L + 1) * LNS

    gold = _gold_score(X, y, trans)
    return (logZ - gold).astype(np.float32)
